# revision 1
# baseline (speedup 1.0000x reference)
"""Trainium2 Bass kernel for a 16-head MHA layer (batch 4, seq 2048, embed 1024).

Sharding: 8 cores; core c handles batch c//2 and query-token half c%2.
Each core receives its batch's x rotated so that its 1024 query tokens sit in
rows 0:1024 (softmax/attention is permutation-invariant over key order, so the
rotation changes nothing mathematically). K/V are computed over the full
sequence on-core, so no collectives are needed. Weights are replicated.

All matmuls run in bf16 (fp32 PSUM accumulation); the exp runs on the scalar
(ACT) engine straight out of PSUM. No max-subtraction is needed: the scaled
scores are ~N(0, 0.33^2), so exp() is safely bounded.
"""

import sys

for _p in ("/opt/trn_rl_repo",):
    if _p not in sys.path:
        sys.path.insert(0, _p)

import numpy as np

import concourse.bass as bass  # noqa: E402
import concourse.mybir as mybir  # noqa: E402
import concourse.tile as tile  # noqa: E402
from concourse import bacc  # noqa: E402
from concourse.masks import make_identity  # noqa: E402

SEQ = 2048
E = 1024
H = 16
D = 64
NQ = 1024  # query tokens per core
N_CORES = 8

F32 = mybir.dt.float32
BF16 = mybir.dt.bfloat16
AF = mybir.ActivationFunctionType


def build_program():
    nc = bacc.Bacc(trn_type="TRN2", target_bir_lowering=False, debug=False)

    x = nc.dram_tensor("x", [SEQ, E], F32, kind="ExternalInput").ap()
    wqkv = nc.dram_tensor("Wqkv", [E, 3 * E], F32, kind="ExternalInput").ap()
    bqkv = nc.dram_tensor("bqkv", [3 * E], F32, kind="ExternalInput").ap()
    wo = nc.dram_tensor("Wo", [E, E], F32, kind="ExternalInput").ap()
    bo = nc.dram_tensor("bo", [E], F32, kind="ExternalInput").ap()
    out = nc.dram_tensor("out", [NQ, E], F32, kind="ExternalOutput").ap()

    ET = E // 128  # 8 e-chunks
    TT = SEQ // 128  # 16 token tiles
    QB = NQ // 512  # 2 query blocks
    KT = SEQ // 128  # 16 key tiles
    HP = H // 2  # 8 head pairs

    with tile.TileContext(nc) as tc:
        _body(nc, tc, x, wqkv, bqkv, wo, bo, out, ET, TT, QB, KT, HP)

    nc.compile()
    return nc


def _body(nc, tc, x, wqkv, bqkv, wo, bo, out, ET, TT, QB, KT, HP):
    from contextlib import ExitStack

    es = ExitStack()
    with es:
        pc = es.enter_context(tc.tile_pool(name="const", bufs=1))
        pat = es.enter_context(tc.tile_pool(name="at", bufs=1))

        # --- constants -------------------------------------------------
        ident = pc.tile([128, 128], BF16, tag="ident")
        make_identity(nc, ident)
        ones128 = pc.tile([128, 128], BF16, tag="ones128")
        nc.vector.memset(ones128, 1.0)
        ident32 = pc.tile([128, 128], F32, tag="ident32")
        make_identity(nc, ident32)

        # bqkv transposed to [128, 24] via PE (chunk c of 128 = column c)
        bq_row = pc.tile([24, 128], F32, tag="bq_row")
        nc.sync.dma_start(out=bq_row, in_=bqkv.rearrange("(c p) -> c p", p=128))
        bqkvT = pc.tile([128, 24], F32, tag="bqkvT")
        with tc.tile_pool(name="ps_misc", bufs=1, space="PSUM") as psm:
            ps_b = psm.tile([128, 24], F32, tag="ps_b")
            nc.tensor.transpose(ps_b, bq_row, ident32[:24, :24])
            nc.vector.tensor_copy(bqkvT, ps_b)

        # attn output (transposed): 8 tiles [128, NQ] bf16; tile p holds heads
        # 2p (rows 0:64) and 2p+1 (rows 64:128)
        AT = [pat.tile([128, NQ], BF16, tag=f"at{p}", name=f"at{p}") for p in range(HP)]

        with (
            tc.tile_pool(name="kqv", bufs=1) as pkqv,
            tc.tile_pool(name="xT", bufs=1) as pxt,
            tc.tile_pool(name="ph1", bufs=2) as p1,
            tc.tile_pool(name="wpan", bufs=3) as pw,
            tc.tile_pool(name="ph3", bufs=3) as p3,
        ):
            KTt = [pkqv.tile([128, SEQ], BF16, tag=f"kt{i}", name=f"ktt{i}") for i in range(ET)]
            QTt = [pkqv.tile([128, NQ], BF16, tag=f"qt{i}", name=f"qtt{i}") for i in range(ET)]
            # V in AV-stationary layout: per key-tile, 8 head pairs of
            # [V_h0 | ones | V_h1] (64+64+64 cols); the shared ones column
            # block makes the denominator come out of the same matmul.
            VO = [pkqv.tile([128, HP, 192], BF16, tag=f"vo{i}", name=f"vo{i}") for i in range(TT)]
            xT = pxt.tile([128, ET, SEQ], BF16, tag="xT")

            def load_panel(pc0, src_w=None):
                src_w = wqkv if src_w is None else src_w
                wp = pw.tile([128, ET, 512], BF16, tag="wp", name=f"wp{id(src_w)}_{pc0}")
                for ee in range(ET):
                    nc.gpsimd.dma_start(
                        out=wp[:, ee, :],
                        in_=src_w[ee * 128 : (ee + 1) * 128, pc0 : pc0 + 512],
                    )
                return wp

            with tc.tile_pool(name="ps_proj", bufs=1, space="PSUM") as ppj:
                from contextlib import ExitStack as _ES2
                _att_es = _ES2()

                def v_chain(wp, panel, tt):
                    # one V-proj output tile -> VO pair layout (+ ones memset)
                    p0 = panel * 4
                    ps = ppj.tile([128, 512], F32, tag="ps")
                    for ee in range(ET):
                        nc.tensor.matmul(
                            ps,
                            lhsT=xT[:, ee, tt * 128 : (tt + 1) * 128],
                            rhs=wp[:, ee, :],
                            start=(ee == 0),
                            stop=(ee == ET - 1),
                        )
                    ps3 = ps.rearrange("p (pr d) -> p pr d", d=128)
                    nc.vector.tensor_copy(VO[tt][:, p0 : p0 + 4, 0:64], ps3[:, :, 0:64])
                    nc.vector.tensor_copy(
                        VO[tt][:, p0 : p0 + 4, 128:192], ps3[:, :, 64:128]
                    )
                    if panel == 0:
                        nc.vector.memset(VO[tt][:, :, 64:128], 1.0)

                def kq_chain(wp, kind, panel, ct, tb):
                    # one K^T/Q^T-proj output tile (+ bias)
                    col0 = E if kind == "k" else 0
                    dst = KTt if kind == "k" else QTt
                    gct = panel * 4 + ct
                    bcol = (col0 + panel * 512 + ct * 128) // 128
                    ps = ppj.tile([128, 512], F32, tag="ps")
                    for ee in range(ET):
                        nc.tensor.matmul(
                            ps,
                            lhsT=wp[:, ee, ct * 128 : (ct + 1) * 128],
                            rhs=xT[:, ee, tb * 512 : (tb + 1) * 512],
                            start=(ee == 0),
                            stop=(ee == ET - 1),
                        )
                    nc.vector.tensor_scalar_add(
                        dst[gct][:, tb * 512 : (tb + 1) * 512],
                        ps,
                        bqkvT[:, bcol : bcol + 1],
                    )

                def attention_block(hp, qb, inner=None):
                    q0 = qb * 512
                    av = [
                        pav.tile([128, 512], F32, tag="av", name=f"av{hp}_{qb}_{i}")
                        for i in range(2)
                    ]
                    for kt in range(KT):
                        if inner is not None and kt < len(inner):
                            inner[kt]()
                        k0 = kt * 128
                        ps_s = pss.tile([128, 1024], F32, tag="ps_s")
                        for i in range(2):
                            r0 = i * 64
                            nc.tensor.matmul(
                                ps_s[:, i * 512 : (i + 1) * 512],
                                lhsT=KTt[hp][r0 : r0 + 64, k0 : k0 + 128],
                                rhs=QTt[hp][r0 : r0 + 64, q0 : q0 + 512],
                                start=True,
                                stop=True,
                            )
                        pt = p3.tile([128, 1024], BF16, tag="pt", bufs=3)
                        nc.scalar.activation(pt, ps_s, AF.Exp, scale=0.125)
                        for i in range(2):
                            # i=0: rows 0:64 = V.T @ P, rows 64:128 = denom
                            # i=1: rows 0:64 = denom, rows 64:128 = V.T @ P
                            nc.tensor.matmul(
                                av[i],
                                lhsT=VO[kt][:, hp, 64 * i : 64 * i + 128],
                                rhs=pt[:, i * 512 : (i + 1) * 512],
                                start=(kt == 0),
                                stop=(kt == KT - 1),
                            )
                    # DVE ops need all inputs at base partition 0 (custom ops
                    # especially); realign the half that sits at rows 64:128 via
                    # a PE select-matmul (identity columns 64:128).
                    for i in range(2):
                        rec = p3.tile([64, 512], F32, tag="rec", bufs=2)
                        r0 = i * 64
                        if i == 0:
                            # AV @ rows 0:64, denom @ rows 64:128 -> move denom
                            av_sb = p3.tile([128, 512], F32, tag="avsb", bufs=2)
                            nc.vector.tensor_copy(av_sb, av[i])
                            dsel = ppj.tile([64, 512], F32, tag="dsel")
                            nc.tensor.matmul(
                                dsel, lhsT=ident32[:, 64:128], rhs=av_sb,
                                start=True, stop=True,
                            )
                            nc.vector.reciprocal_approx_fast(rec, dsel)
                            nc.vector.tensor_mul(
                                AT[hp][r0 : r0 + 64, q0 : q0 + 512],
                                av[i][0:64, :],
                                rec,
                            )
                        else:
                            # denom @ rows 0:64, AV @ rows 64:128 -> move AV
                            # (bf16 select: AT is bf16 anyway, so no extra loss)
                            av_sb = p3.tile([128, 512], BF16, tag="avsb2", bufs=2)
                            nc.vector.tensor_copy(av_sb, av[i])
                            nc.vector.reciprocal_approx_fast(rec, av[i][0:64, :])
                            asel = ppj.tile([64, 512], F32, tag="dsel")
                            nc.tensor.matmul(
                                asel, lhsT=ident[:, 64:128], rhs=av_sb,
                                start=True, stop=True,
                            )
                            nc.vector.tensor_mul(
                                AT[hp][r0 : r0 + 64, q0 : q0 + 512],
                                asel,
                                rec,
                            )

                # --- upfront: x -> xT transposes interleaved with K0/Q0
                # proj chains per 512-token block (chains only need their own
                # token block of xT). x tiles are prefetched ahead of the
                # weight-panel DMAs so the transpose pipeline starts early.
                def dma_x(tt):
                    xb = p1.tile([128, E], BF16, tag="xb", bufs=4, name=f"xb{tt}")
                    nc.gpsimd.dma_start(out=xb, in_=x[tt * 128 : (tt + 1) * 128, :])
                    return xb

                xq = [dma_x(tt) for tt in range(4)]
                wp_k0 = load_panel(E)
                wp_q0 = None
                with tc.tile_pool(name="ps_tr", bufs=2, space="PSUM") as ptr:
                    for tb in range(4):
                        for tt in range(4 * tb, 4 * tb + 4):
                            xb = xq.pop(0)
                            for ee in range(ET):
                                ps = ptr.tile([128, 128], BF16, tag="ps")
                                nc.tensor.transpose(
                                    ps, xb[:, ee * 128 : (ee + 1) * 128], ident
                                )
                                nc.vector.tensor_copy(
                                    xT[:, ee, tt * 128 : (tt + 1) * 128], ps
                                )
                            if tt + 4 < TT:
                                xq.append(dma_x(tt + 4))
                            if tt == 5:
                                wp_q0 = load_panel(0)
                        kq_chain(wp_k0, "k", 0, 0, tb)
                        if tb >= 2:
                            kq_chain(wp_q0, "q", 0, 0, tb - 2)

                pss = _att_es.enter_context(
                    tc.tile_pool(name="ps_s", bufs=2, space="PSUM")
                )
                pav = _att_es.enter_context(
                    tc.tile_pool(name="ps_av", bufs=2, space="PSUM")
                )
                # --- V panel 0 feeds hp0/qb0 just-in-time; the rest of the
                # projection work is sprinkled between attention blocks so it
                # hides under the exp-bound attention pipeline. Chains are
                # ordered/paced so every tile is written before the block that
                # reads it: K1/Q1 coltile ct feeds attention hp=4+ct (block
                # 2*(4+ct)); V panel 1 feeds all of hp4-7 (block 8).
                wp_v0 = load_panel(2 * E)
                wp_k1 = load_panel(E + 512)
                wp_q1 = load_panel(512)
                inner0 = [
                    (lambda t=tt: v_chain(wp_v0, 0, t)) for tt in range(TT)
                ]

                wp_v1 = [None]
                wp_k1 = [None]
                wp_q1 = [None]
                # deadline-ordered work queue; K/Q coltile ct of panel p feeds
                # attention pair hp = 4*p + ct, i.e. block 2*hp; V panel 1
                # feeds all of hp4-7 (block 8).
                deferred = (
                    [
                        ch
                        for ct in (1, 2, 3)
                        for ch in (
                            [
                                (lambda c=ct, t=tb: kq_chain(wp_k0, "k", 0, c, t))
                                for tb in range(4)
                            ]
                            + [
                                (lambda c=ct, t=tb: kq_chain(wp_q0, "q", 0, c, t))
                                for tb in range(2)
                            ]
                        )
                    ]
                    + [(lambda t=tt: v_chain(wp_v1[0], 1, t)) for tt in range(TT)]
                    + [(lambda t=tb: kq_chain(wp_k1[0], "k", 1, 0, t)) for tb in range(4)]
                    + [(lambda t=tb: kq_chain(wp_q1[0], "q", 1, 0, t)) for tb in range(2)]
                    + [
                        ch
                        for ct in (1, 2, 3)
                        for ch in (
                            [
                                (lambda c=ct, t=tb: kq_chain(wp_k1[0], "k", 1, c, t))
                                for tb in range(4)
                            ]
                            + [
                                (lambda c=ct, t=tb: kq_chain(wp_q1[0], "q", 1, c, t))
                                for tb in range(2)
                            ]
                        )
                    ]
                )
                # chains emitted at the START of blocks 1..15 (index 0 = block 1)
                plan = [5, 5, 5, 5, 5, 5, 5, 5, 4, 4, 4, 3, 3, 0, 0]
                assert sum(plan) == len(deferred)

                # output projection pieces (wob/boB built during block 14,
                # out-proj chains sprinkled after their token columns finish)
                wob = [None, None]
                boB = pc.tile([128, E], F32, tag="boB")

                def outproj_setup():
                    wob[0] = load_panel(0, src_w=wo)
                    wob[1] = load_panel(512, src_w=wo)
                    bv_rep = p1.tile([128, ET, 128], BF16, tag="bvrep", bufs=1)
                    for ee in range(ET):
                        nc.vector.tensor_scalar_mul(
                            bv_rep[:, ee, :], ones128, bqkvT[:, 16 + ee : 17 + ee]
                        )
                    boT = p1.tile([128, E], F32, tag="boT", bufs=1)
                    bo_bcast = bass.AP(
                        tensor=bo.tensor, offset=bo.offset, ap=[[0, 128]] + bo.ap
                    )
                    nc.gpsimd.dma_start(out=boT, in_=bo_bcast)
                    for half in range(2):
                        c0 = half * 512
                        psb = ppj.tile([128, 512], F32, tag="ps")
                        for ee in range(ET):
                            nc.tensor.matmul(
                                psb,
                                lhsT=bv_rep[:, ee, :],
                                rhs=wob[half][:, ee, :],
                                start=(ee == 0),
                                stop=(ee == ET - 1),
                            )
                        nc.vector.tensor_add(
                            boB[:, c0 : c0 + 512], psb, boT[:, c0 : c0 + 512]
                        )

                def outproj_chain(tt, half):
                    c0 = half * 512
                    ps = ppj.tile([128, 512], F32, tag="ps")
                    for ee in range(ET):
                        nc.tensor.matmul(
                            ps,
                            lhsT=AT[ee][:, tt * 128 : (tt + 1) * 128],
                            rhs=wob[half][:, ee, :],
                            start=(ee == 0),
                            stop=(ee == ET - 1),
                        )
                    osb = p3.tile([128, 512], F32, tag="osb", bufs=2)
                    nc.vector.tensor_add(osb, ps, boB[:, c0 : c0 + 512])
                    nc.sync.dma_start(
                        out=out[tt * 128 : (tt + 1) * 128, c0 : c0 + 512], in_=osb
                    )

                blocks = [(hp, qb) for hp in range(HP) for qb in range(QB)]
                di = 0
                for b, (hp, qb) in enumerate(blocks):
                    if b == 0:
                        attention_block(hp, qb, inner=inner0)
                        continue
                    if b == 1:
                        wp_v1[0] = load_panel(2 * E + 512)
                    if b == 5:
                        wp_k1[0] = load_panel(E + 512)
                    if b == 6:
                        wp_q1[0] = load_panel(512)
                    for _ in range(plan[b - 1]):
                        deferred[di]()
                        di += 1
                    if b == 14:
                        outproj_setup()
                    attention_block(hp, qb)
                    if b == 14:
                        # all heads' qb=0 columns are complete
                        for tt in range(4):
                            for half in range(2):
                                outproj_chain(tt, half)
                assert di == len(deferred)
                for tt in range(4, 8):
                    for half in range(2):
                        outproj_chain(tt, half)
                _att_es.close()


_NC = None


def _get_program():
    global _NC
    if _NC is None:
        _NC = build_program()
    return _NC


def make_in_maps(x, Wqkv, bqkv, Wo, bo):
    w = {
        "Wqkv": np.ascontiguousarray(np.asarray(Wqkv, np.float32)),
        "bqkv": np.ascontiguousarray(np.asarray(bqkv, np.float32)),
        "Wo": np.ascontiguousarray(np.asarray(Wo, np.float32)),
        "bo": np.ascontiguousarray(np.asarray(bo, np.float32)),
    }
    x = np.asarray(x, np.float32)
    in_maps = []
    for c in range(N_CORES):
        b, s = divmod(c, 2)
        xb = x[b]
        if s == 1:
            xb = np.roll(xb, -NQ, axis=0)
        in_maps.append({"x": np.ascontiguousarray(xb), **w})
    return in_maps


def gather_out(results):
    out = np.empty((4, SEQ, E), np.float32)
    for c in range(N_CORES):
        b, s = divmod(c, 2)
        out[b, s * NQ : (s + 1) * NQ] = results[c]["out"]
    return out


def kernel(x, Wqkv, bqkv, Wo, bo):
    from concourse.bass_utils import run_bass_kernel_spmd

    nc = _get_program()
    in_maps = make_in_maps(x, Wqkv, bqkv, Wo, bo)
    res = run_bass_kernel_spmd(nc, in_maps, core_ids=list(range(N_CORES)))
    return gather_out(res.results)



# revision 8
# speedup vs baseline: 1.3751x; 1.3751x over previous
"""Trainium2 Bass kernel for a 16-head MHA layer (batch 4, seq 2048, embed 1024).

Sharding: 8 cores; core c handles batch c//2 and query-token half c%2, with the
core's x rotated so its 1024 query tokens sit in rows 0:1024 (softmax is
permutation-invariant over key order). K/V cover the full sequence on-core; no
collectives. Weights replicated.

Numerics: every hot matmul runs in fp8e4 DoubleRow (2 contraction slots per
matmul at 0.5 cycles/output-column), with the residual-pair trick soaking up
fp8 quantization error wherever a slot is free:
 - QK projections contract (W_hi, W_lo) fp8 pairs of 32*Wqkv (scaled out of
   the fp8-subnormal range; host-prepared).
 - Scores contract d=64 with slots (Q_hi, Q_lo) -- the Q store residual --
   against a stride-0-duplicated K.
 - AV contracts slots ([ones/32|V_hi], [ones/32|V_lo]) against stride-0-
   duplicated P, so each matmul emits denominator/16 in rows 0:64 and the
   V-residual-corrected AV in rows 64:128.
 - V projection and the output projection run in bf16 (x^T is uploaded twice:
   fp8 for QK chains, bf16 for V chains; out-proj reads bf16 AT tiles).
 - K bias is dropped (softmax-invariant); V bias + output bias fold into a
   host-precomputed boB = b_v @ Wo + bo; Q bias (x32) is added on-chip.
 - exp: mostly exact on the ACT engine (fp8 out, scale folded); a tunable set
   of key-tiles runs a one-instruction DVE Schraudolph (int8 bit pattern) exp
   to split the exp load across both engines.
"""

import sys

for _p in ("/opt/trn_rl_repo",):
    if _p not in sys.path:
        sys.path.insert(0, _p)

import numpy as np

import concourse.bass as bass  # noqa: E402
import concourse.mybir as mybir  # noqa: E402
import concourse.tile as tile  # noqa: E402
from concourse import bacc  # noqa: E402

SEQ = 2048
E = 1024
H = 16
D = 64
NQ = 1024  # query tokens per core
N_CORES = 8

ET = E // 128  # 8 e-chunks
TT = SEQ // 128  # 16 key/token tiles
HP = H // 2  # 8 head pairs
QB = NQ // 512  # 2 query blocks

F32 = mybir.dt.float32
BF16 = mybir.dt.bfloat16
FP8 = mybir.dt.float8e4
I8 = mybir.dt.int8
AF = mybir.ActivationFunctionType
PM = mybir.MatmulPerfMode

# exp(s_fp8 * SC) == exp(s_true * 0.125); fp8 QK weights carry 32x each
SC = 0.125 / 1024.0
# Schraudolph: int8 bits = round(s_fp8*C1 + C2) reinterpreted as fp8e4
C1 = 8 * 1.4426950408889634 * SC
C2 = 56.0 - 8 * 0.043


def _dve_kts(b):
    if b in (0, 8):  # these blocks' DVE is busy with inner V copies
        return ()
    return (2, 6, 10, 12)


def build_program():
    nc = bacc.Bacc(trn_type="TRN2", target_bir_lowering=False, debug=False)

    xT8 = nc.dram_tensor("xT8", [E, SEQ], FP8, kind="ExternalInput").ap()
    xTb = nc.dram_tensor("xTb", [E, SEQ], BF16, kind="ExternalInput").ap()
    wqk_hi = nc.dram_tensor("wqk_hi", [E, 2 * E], FP8, kind="ExternalInput").ap()
    wqk_lo = nc.dram_tensor("wqk_lo", [E, 2 * E], FP8, kind="ExternalInput").ap()
    wv_b = nc.dram_tensor("wv_b", [E, E], BF16, kind="ExternalInput").ap()
    wo_b = nc.dram_tensor("wo_b", [E, E], BF16, kind="ExternalInput").ap()
    bqT = nc.dram_tensor("bqT", [128, ET], F32, kind="ExternalInput").ap()
    boB = nc.dram_tensor("boB", [E], F32, kind="ExternalInput").ap()
    out = nc.dram_tensor("out", [NQ, E], F32, kind="ExternalOutput").ap()

    with tile.TileContext(nc) as tc:
        _body(nc, tc, xT8, xTb, wqk_hi, wqk_lo, wv_b, wo_b, bqT, boB, out)

    nc.compile()
    return nc


def _body(nc, tc, xT8_d, xTb_d, wqkh_d, wqkl_d, wvb_d, wob_d, bqT_d, boB_d, out):
    from contextlib import ExitStack

    es = ExitStack()
    with es:
        pc = es.enter_context(tc.tile_pool(name="const", bufs=1))
        pat = es.enter_context(tc.tile_pool(name="at", bufs=1))
        pkqv = es.enter_context(tc.tile_pool(name="kqv", bufs=1))
        pwf = es.enter_context(tc.tile_pool(name="wpan8", bufs=4))
        pwb = es.enter_context(tc.tile_pool(name="wpanb", bufs=2))
        pP = es.enter_context(tc.tile_pool(name="pP", bufs=2))
        p3 = es.enter_context(tc.tile_pool(name="p3", bufs=2))
        ppj = es.enter_context(tc.tile_pool(name="ps_proj", bufs=2, space="PSUM"))

        # --- persistent tensors -------------------------------------------
        xT8 = pkqv.tile([128, ET, SEQ], FP8, tag="xT8")
        K8 = pkqv.tile([128, ET, SEQ], FP8, tag="K8")
        Q8 = pkqv.tile([128, ET, 2, NQ], FP8, tag="Q8")
        # VO[kt, hp, slot, 128]: slot 2h+r = [ones/32 | V_{hi,lo} of head h]
        VO = pkqv.tile([128, TT, HP, 4, 128], FP8, tag="VO")
        AT4 = [
            pat.tile([128, 2, NQ], BF16, tag=f"at{j}", name=f"at{j}")
            for j in range(4)
        ]
        bqT = pc.tile([128, ET], F32, tag="bqT")
        boB = pc.tile([128, E], F32, tag="boB")

        # xTb streams in per-token-tile slices (v_chain tt only reads its own)
        pxtb = es.enter_context(tc.tile_pool(name="xtb", bufs=4))

        # --- input DMAs (sync queue, HWDGE) -------------------------------
        xT8_r = xT8_d.rearrange("(c p) t -> p c t", p=128)
        xTb_r = xTb_d.rearrange("(c p) t -> p c t", p=128)

        def load_panel(pool, src_w, pc0, name, dt, tag):
            wp = pool.tile([128, ET, 512], dt, tag=tag, name=name)
            nc.sync.dma_start(
                out=wp,
                in_=src_w.rearrange("(c p) n -> p c n", p=128)[
                    :, :, pc0 : pc0 + 512
                ],
            )
            return wp

        wp_v0 = load_panel(pwb, wvb_d, 0, "wpv0", BF16, "wpb")
        nc.sync.dma_start(out=xT8, in_=xT8_r)
        wp_k0h = load_panel(pwf, wqkh_d, E, "wpk0h", FP8, "wpf")
        wp_k0l = load_panel(pwf, wqkl_d, E, "wpk0l", FP8, "wpf")
        wp_q0h = load_panel(pwf, wqkh_d, 0, "wpq0h", FP8, "wpf")
        wp_q0l = load_panel(pwf, wqkl_d, 0, "wpq0l", FP8, "wpf")
        nc.sync.dma_start(out=bqT, in_=bqT_d)
        boB_bcast = bass.AP(
            tensor=boB_d.tensor, offset=boB_d.offset, ap=[[0, 128]] + boB_d.ap
        )
        nc.sync.dma_start(out=boB, in_=boB_bcast)

        # --- VO ones blocks (Pool engine) ---------------------------------
        for kt in range(TT):
            nc.gpsimd.memset(VO[:, kt, :, :, 0:64], 0.03125)

        # --- chain builders ------------------------------------------------
        def kq_chain(wph, wpl, kind, panel, ct, tb):
            """One K/Q projection coltile with W-residual: out [128d, 512t]."""
            gct = panel * 4 + ct
            ps = ppj.tile([128, 512], F32, tag="ps")
            first = True
            for wp in (wph, wpl):
                for eh in range(4):
                    nc.tensor.matmul(
                        ps,
                        lhsT=wp[:, 2 * eh : 2 * eh + 2, ct * 128 : (ct + 1) * 128],
                        rhs=xT8[:, 2 * eh : 2 * eh + 2, tb * 512 : (tb + 1) * 512],
                        start=first,
                        stop=(wp is wpl and eh == 3),
                        perf_mode=PM.DoubleRow,
                    )
                    first = False
            if kind == "k":
                nc.vector.tensor_copy(
                    K8[:, gct, tb * 512 : (tb + 1) * 512], ps
                )
            else:
                nc.vector.tensor_scalar_add(
                    Q8[:, gct, 0, tb * 512 : (tb + 1) * 512],
                    ps,
                    bqT[:, gct : gct + 1],
                )
                nc.vector.scalar_tensor_tensor(
                    out=Q8[:, gct, 1, tb * 512 : (tb + 1) * 512],
                    in0=ps,
                    scalar=bqT[:, gct : gct + 1],
                    in1=Q8[:, gct, 0, tb * 512 : (tb + 1) * 512],
                    op0=mybir.AluOpType.add,
                    op1=mybir.AluOpType.subtract,
                )

        def v_chain(wvp, panel, tt):
            """One V projection token tile (bf16): out [128 tok, 512 vcol],
            split into fp8 hi/lo pairs in the VO windows of hp 4p..4p+3."""
            xtb = pxtb.tile([128, ET, 128], BF16, tag="xtb", name=f"xtb{panel}_{tt}")
            nc.sync.dma_start(
                out=xtb, in_=xTb_r[:, :, tt * 128 : (tt + 1) * 128]
            )
            ps = ppj.tile([128, 512], F32, tag="ps")
            for ee in range(ET):
                nc.tensor.matmul(
                    ps,
                    lhsT=xtb[:, ee, :],
                    rhs=wvp[:, ee, :],
                    start=(ee == 0),
                    stop=(ee == ET - 1),
                )
            ps3 = ps.rearrange("p (hl hd d) -> p hl hd d", hd=2, d=64)
            p0 = panel * 4
            vo_b = VO[:, tt, p0, 0, :]
            hi = bass.AP(
                tensor=vo_b.tensor,
                offset=vo_b.offset + 64,
                ap=[vo_b.ap[0], [512, 4], [256, 2], [1, 64]],
            )
            lo = bass.AP(
                tensor=vo_b.tensor,
                offset=vo_b.offset + 128 + 64,
                ap=[vo_b.ap[0], [512, 4], [256, 2], [1, 64]],
            )
            nc.vector.tensor_copy(hi, ps3)
            nc.vector.tensor_sub(lo, ps3, hi)

        # --- output projection (bf16) --------------------------------------
        wop = [None, None]

        def outproj_chain(tt, half):
            c0 = half * 512
            ps = ppj.tile([128, 512], F32, tag="ps")
            for j in range(4):
                for i in range(2):
                    nc.tensor.matmul(
                        ps,
                        lhsT=AT4[j][:, i, tt * 128 : (tt + 1) * 128],
                        rhs=wop[half][:, 2 * j + i, :],
                        start=(j == 0 and i == 0),
                        stop=(j == 3 and i == 1),
                    )
            osb = p3.tile([128, 512], F32, tag="osb", bufs=2)
            nc.vector.scalar_tensor_tensor(
                out=osb,
                in0=ps,
                scalar=1.0 / 512.0,
                in1=boB[:, c0 : c0 + 512],
                op0=mybir.AluOpType.mult,
                op1=mybir.AluOpType.add,
            )
            nc.sync.dma_start(
                out=out[tt * 128 : (tt + 1) * 128, c0 : c0 + 512], in_=osb
            )

        # --- upfront chains -------------------------------------------------
        for tt in range(6):
            v_chain(wp_v0, 0, tt)
        for tb in range(4):
            kq_chain(wp_k0h, wp_k0l, "k", 0, 0, tb)
        kq_chain(wp_q0h, wp_q0l, "q", 0, 0, 0)

        # --- deferred chain queue (deadline, fn), emitted 1 per kt slot ----
        panels = {}
        deferred = []

        def defer(dl, fn):
            deferred.append((dl, fn))

        for ct in (1, 2, 3):
            for tb in range(4):
                defer(2 * ct, lambda c=ct, t=tb: kq_chain(wp_k0h, wp_k0l, "k", 0, c, t))
        for g in range(1, 4):
            for qb in range(2):
                defer(2 * g + qb, lambda c=g, q=qb: kq_chain(wp_q0h, wp_q0l, "q", 0, c, q))
        defer(1, lambda: kq_chain(wp_q0h, wp_q0l, "q", 0, 0, 1))
        # v1 chains: 6 early (deadlines 6-7), 10 inner in block 8
        for i in range(6):
            defer(6 + i // 3, lambda t=i: v_chain(panels["v1"], 1, t))
        for ct in range(4):
            for tb in range(4):
                defer(8 + 2 * ct, lambda c=ct, t=tb: kq_chain(panels["k1h"], panels["k1l"], "k", 1, c, t))
        for g in range(4):
            for qb in range(2):
                defer(8 + 2 * g + qb, lambda c=g, q=qb: kq_chain(panels["q1h"], panels["q1l"], "q", 1, c, q))
        deferred.sort(key=lambda t: t[0])

        # --- attention ------------------------------------------------------
        att_es = ExitStack()
        pss = att_es.enter_context(tc.tile_pool(name="ps_s", bufs=2, space="PSUM"))
        pav = att_es.enter_context(tc.tile_pool(name="ps_av", bufs=1, space="PSUM"))

        def normalize(hp, qb, av):
            q0 = qb * 512
            rec = p3.tile([64, 1024], F32, tag="rec", bufs=2)
            nc.vector.reciprocal_approx_fast(rec, av[0:64, :])
            j, i = hp // 2, hp % 2
            nc.vector.tensor_mul(
                AT4[j][0:64, i, q0 : q0 + 512], av[64:128, 0:512], rec[:, 0:512]
            )
            nc.vector.tensor_mul(
                AT4[j][64:128, i, q0 : q0 + 512],
                av[64:128, 512:1024],
                rec[:, 512:1024],
            )

        def attention_block(b, hp, qb, inner):
            q0 = qb * 512
            av = pav.tile([128, 1024], F32, tag="av", name=f"av{b}")
            Pt = [None]
            dve_kts = _dve_kts(b)
            for kt in range(TT):
                for fn in inner.get(kt, ()):
                    fn()
                ktm = kt % 2
                if ktm == 0:
                    Pt[0] = pP.tile(
                        [128, 2, 2, 512], FP8, tag="P", name=f"P{b}_{kt}"
                    )
                P = Pt[0]
                ps = pss.tile([128, 1024], F32, tag="ps_s")
                for h in range(2):
                    r0 = 64 * h
                    kbase = K8[r0 : r0 + 64, hp, kt * 128 : (kt + 1) * 128]
                    lhsT = bass.AP(
                        tensor=kbase.tensor,
                        offset=kbase.offset,
                        ap=[kbase.ap[0], [0, 2], kbase.ap[-1]],
                    )
                    nc.tensor.matmul(
                        ps[:, h * 512 : (h + 1) * 512],
                        lhsT=lhsT,
                        rhs=Q8[r0 : r0 + 64, hp, :, q0 : q0 + 512],
                        start=True,
                        stop=True,
                        perf_mode=PM.DoubleRow,
                    )
                if kt in dve_kts:
                    nc.vector.tensor_scalar(
                        out=P[:, :, ktm, :].bitcast(I8),
                        in0=ps,
                        scalar1=C1,
                        scalar2=C2,
                        op0=mybir.AluOpType.mult,
                        op1=mybir.AluOpType.add,
                    )
                else:
                    nc.scalar.activation(P[:, :, ktm, :], ps, AF.Exp, scale=SC)
                for h in range(2):
                    pslot = P[:, h, ktm, :]
                    rhs = bass.AP(
                        tensor=pslot.tensor,
                        offset=pslot.offset,
                        ap=[pslot.ap[0], [0, 2], pslot.ap[-1]],
                    )
                    nc.tensor.matmul(
                        av[:, h * 512 : (h + 1) * 512],
                        lhsT=VO[:, kt, hp, 2 * h : 2 * h + 2, :],
                        rhs=rhs,
                        start=(kt == 0),
                        stop=(kt == TT - 1),
                        perf_mode=PM.DoubleRow,
                    )
            return av

        blocks = [(hp, qb) for hp in range(HP) for qb in range(QB)]
        av_prev = None
        di = 0
        for b, (hp, qb) in enumerate(blocks):
            if av_prev is not None:
                normalize(*av_prev)
                av_prev = None
            if b == 3:
                panels["v1"] = load_panel(pwb, wvb_d, 512, "wpv1", BF16, "wpb")
            if b == 5:
                panels["k1h"] = load_panel(pwf, wqkh_d, E + 512, "wpk1h", FP8, "wpf")
                panels["k1l"] = load_panel(pwf, wqkl_d, E + 512, "wpk1l", FP8, "wpf")
            if b == 6:
                panels["q1h"] = load_panel(pwf, wqkh_d, 512, "wpq1h", FP8, "wpf")
                panels["q1l"] = load_panel(pwf, wqkl_d, 512, "wpq1l", FP8, "wpf")
            if b == 13:
                # reuse the bf16 panel pool (rotation deps make this safe)
                wop[0] = load_panel(pwb, wob_d, 0, "wpo0", BF16, "wpb")
                wop[1] = load_panel(pwb, wob_d, 512, "wpo1", BF16, "wpb")

            # schedule this block's inner work: 1 item per kt slot
            inner = {}
            slot_fns = []
            if b == 0:
                slot_fns += [
                    (lambda t=tt: v_chain(wp_v0, 0, t)) for tt in range(6, TT)
                ]
            if b == 8:
                slot_fns += [
                    (lambda t=tt: v_chain(panels["v1"], 1, t))
                    for tt in range(6, TT)
                ]
            while di < len(deferred) and deferred[di][0] <= b + 1:
                slot_fns.append(deferred[di][1])
                di += 1
            if b == 15:
                for tt in range(4):
                    for half in range(2):
                        slot_fns.append(lambda t=tt, hf=half: outproj_chain(t, hf))
            start = 2 if b > 0 else 0
            for i, fn in enumerate(slot_fns):
                inner.setdefault(min(start + i, TT - 1), []).append(fn)

            av = attention_block(b, hp, qb, inner)
            av_prev = (hp, qb, av)
            if b >= 14:
                # normalize immediately: block 15's inner outproj chains (qb0)
                # and the tail (qb1) read these AT4 columns
                normalize(*av_prev)
                av_prev = None
        assert di == len(deferred), (di, len(deferred))
        for tt in range(4, 8):
            for half in range(2):
                outproj_chain(tt, half)
        att_es.close()


_NC = None


def _get_program():
    global _NC
    if _NC is None:
        _NC = build_program()
    return _NC


def make_in_maps(x, Wqkv, bqkv, Wo, bo):
    np8 = mybir.dt.np(FP8)
    npb = mybir.dt.np(BF16)
    Wqkv = np.asarray(Wqkv, np.float32)
    Wo = np.asarray(Wo, np.float32)
    bqkv = np.asarray(bqkv, np.float32)
    bo = np.asarray(bo, np.float32)
    wqk32 = 32.0 * Wqkv[:, 0 : 2 * E]
    wqk_hi = wqk32.astype(np8)
    wqk_lo = (wqk32 - wqk_hi.astype(np.float32)).astype(np8)
    w = {
        "wqk_hi": np.ascontiguousarray(wqk_hi),
        "wqk_lo": np.ascontiguousarray(wqk_lo),
        "wv_b": np.ascontiguousarray((32.0 * Wqkv[:, 2 * E :]).astype(npb)),
        "wo_b": np.ascontiguousarray(Wo.astype(npb)),
        "bqT": np.ascontiguousarray(
            (32.0 * bqkv[0:E]).reshape(ET, 128).T.astype(np.float32)
        ),
        "boB": np.ascontiguousarray((bqkv[2 * E :] @ Wo + bo).astype(np.float32)),
    }
    x = np.asarray(x, np.float32)
    in_maps = []
    for c in range(N_CORES):
        b, s = divmod(c, 2)
        xb = x[b]
        if s == 1:
            xb = np.roll(xb, -NQ, axis=0)
        xt = np.ascontiguousarray(xb.T)
        in_maps.append(
            {
                "xT8": np.ascontiguousarray(xt.astype(np8)),
                "xTb": np.ascontiguousarray(xt.astype(npb)),
                **w,
            }
        )
    return in_maps


def gather_out(results):
    out = np.empty((4, SEQ, E), np.float32)
    for c in range(N_CORES):
        b, s = divmod(c, 2)
        out[b, s * NQ : (s + 1) * NQ] = results[c]["out"]
    return out


def kernel(x, Wqkv, bqkv, Wo, bo):
    from concourse.bass_utils import run_bass_kernel_spmd

    nc = _get_program()
    in_maps = make_in_maps(x, Wqkv, bqkv, Wo, bo)
    res = run_bass_kernel_spmd(nc, in_maps, core_ids=list(range(N_CORES)))
    return gather_out(res.results)


# revision 9
# speedup vs baseline: 1.3843x; 1.0067x over previous
"""Trainium2 Bass kernel for a 16-head MHA layer (batch 4, seq 2048, embed 1024).

Sharding: 8 cores; core c handles batch c//2 and query-token half c%2, with the
core's x rotated so its 1024 query tokens sit in rows 0:1024 (softmax is
permutation-invariant over key order). K/V cover the full sequence on-core; no
collectives. Weights replicated.

Numerics: every hot matmul runs in fp8e4 DoubleRow (2 contraction slots per
matmul at 0.5 cycles/output-column), with the residual-pair trick soaking up
fp8 quantization error wherever a slot is free:
 - QK projections contract (W_hi, W_lo) fp8 pairs of 32*Wqkv (scaled out of
   the fp8-subnormal range; host-prepared).
 - Scores contract d=64 with slots (Q_hi, Q_lo) -- the Q store residual --
   against a stride-0-duplicated K.
 - AV contracts slots ([ones/32|V_hi], [ones/32|V_lo]) against stride-0-
   duplicated P, so each matmul emits denominator/16 in rows 0:64 and the
   V-residual-corrected AV in rows 64:128.
 - V projection and the output projection run in bf16 (x^T is uploaded twice:
   fp8 for QK chains, bf16 for V chains; out-proj reads bf16 AT tiles).
 - K bias is dropped (softmax-invariant); V bias + output bias fold into a
   host-precomputed boB = b_v @ Wo + bo; Q bias (x32) is added on-chip.
 - exp: mostly exact on the ACT engine (fp8 out, scale folded); a tunable set
   of key-tiles runs a one-instruction DVE Schraudolph (int8 bit pattern) exp
   to split the exp load across both engines.
"""

import sys

for _p in ("/opt/trn_rl_repo",):
    if _p not in sys.path:
        sys.path.insert(0, _p)

import numpy as np

import concourse.bass as bass  # noqa: E402
import concourse.mybir as mybir  # noqa: E402
import concourse.tile as tile  # noqa: E402
from concourse import bacc  # noqa: E402

SEQ = 2048
E = 1024
H = 16
D = 64
NQ = 1024  # query tokens per core
N_CORES = 8

ET = E // 128  # 8 e-chunks
TT = SEQ // 128  # 16 key/token tiles
HP = H // 2  # 8 head pairs
QB = NQ // 512  # 2 query blocks

F32 = mybir.dt.float32
BF16 = mybir.dt.bfloat16
FP8 = mybir.dt.float8e4
I8 = mybir.dt.int8
AF = mybir.ActivationFunctionType
PM = mybir.MatmulPerfMode

# exp(s_fp8 * SC) == exp(s_true * 0.125); fp8 QK weights carry 32x each
SC = 0.125 / 1024.0
# Schraudolph: int8 bits = round(s_fp8*C1 + C2) reinterpreted as fp8e4
C1 = 8 * 1.4426950408889634 * SC
C2 = 56.0 - 8 * 0.043


def _dve_kts(b):
    if b in (0, 8):  # these blocks' DVE is busy with inner V copies
        return ()
    return (2, 6, 10, 12)


def build_program():
    nc = bacc.Bacc(trn_type="TRN2", target_bir_lowering=False, debug=False)

    xT8 = nc.dram_tensor("xT8", [E, SEQ], FP8, kind="ExternalInput").ap()
    xTb = nc.dram_tensor("xTb", [E, SEQ], BF16, kind="ExternalInput").ap()
    wqk_hi = nc.dram_tensor("wqk_hi", [E, 2 * E], FP8, kind="ExternalInput").ap()
    wqk_lo = nc.dram_tensor("wqk_lo", [E, 2 * E], FP8, kind="ExternalInput").ap()
    wv_b = nc.dram_tensor("wv_b", [E, E], BF16, kind="ExternalInput").ap()
    wo_b = nc.dram_tensor("wo_b", [E, E], BF16, kind="ExternalInput").ap()
    bqT = nc.dram_tensor("bqT", [128, ET], F32, kind="ExternalInput").ap()
    boB = nc.dram_tensor("boB", [E], F32, kind="ExternalInput").ap()
    out = nc.dram_tensor("out", [NQ, E], F32, kind="ExternalOutput").ap()

    with tile.TileContext(nc) as tc:
        _body(nc, tc, xT8, xTb, wqk_hi, wqk_lo, wv_b, wo_b, bqT, boB, out)

    nc.compile()
    return nc


def _body(nc, tc, xT8_d, xTb_d, wqkh_d, wqkl_d, wvb_d, wob_d, bqT_d, boB_d, out):
    from contextlib import ExitStack

    es = ExitStack()
    with es:
        pc = es.enter_context(tc.tile_pool(name="const", bufs=1))
        pat = es.enter_context(tc.tile_pool(name="at", bufs=1))
        pkqv = es.enter_context(tc.tile_pool(name="kqv", bufs=1))
        pwf = es.enter_context(tc.tile_pool(name="wpan8", bufs=4))
        pwb = es.enter_context(tc.tile_pool(name="wpanb", bufs=2))
        pP = es.enter_context(tc.tile_pool(name="pP", bufs=2))
        p3 = es.enter_context(tc.tile_pool(name="p3", bufs=2))
        ppj = es.enter_context(tc.tile_pool(name="ps_proj", bufs=2, space="PSUM"))

        # --- persistent tensors -------------------------------------------
        xT8 = pkqv.tile([128, ET, SEQ], FP8, tag="xT8")
        K8 = pkqv.tile([128, ET, SEQ], FP8, tag="K8")
        Q8 = pkqv.tile([128, ET, 2, NQ], FP8, tag="Q8")
        # VO[kt, hp, slot, 128]: slot 2h+r = [ones/32 | V_{hi,lo} of head h]
        VO = pkqv.tile([128, TT, HP, 4, 128], FP8, tag="VO")
        AT4 = [
            pat.tile([128, 2, NQ], BF16, tag=f"at{j}", name=f"at{j}")
            for j in range(4)
        ]
        bqT = pc.tile([128, ET], F32, tag="bqT")
        boB = pc.tile([128, E], F32, tag="boB")

        # xTb streams in per-token-tile slices (v_chain tt only reads its own)
        pxtb = es.enter_context(tc.tile_pool(name="xtb", bufs=4))

        # --- input DMAs (sync queue, HWDGE) -------------------------------
        xT8_r = xT8_d.rearrange("(c p) t -> p c t", p=128)
        xTb_r = xTb_d.rearrange("(c p) t -> p c t", p=128)

        def load_panel(pool, src_w, pc0, name, dt, tag):
            wp = pool.tile([128, ET, 512], dt, tag=tag, name=name)
            nc.sync.dma_start(
                out=wp,
                in_=src_w.rearrange("(c p) n -> p c n", p=128)[
                    :, :, pc0 : pc0 + 512
                ],
            )
            return wp

        wp_v0 = load_panel(pwb, wvb_d, 0, "wpv0", BF16, "wpb")
        for tb in range(4):
            nc.sync.dma_start(
                out=xT8[:, :, tb * 512 : (tb + 1) * 512],
                in_=xT8_r[:, :, tb * 512 : (tb + 1) * 512],
            )
            if tb == 0:
                wp_k0h = load_panel(pwf, wqkh_d, E, "wpk0h", FP8, "wpf")
                wp_k0l = load_panel(pwf, wqkl_d, E, "wpk0l", FP8, "wpf")
            if tb == 1:
                wp_q0h = load_panel(pwf, wqkh_d, 0, "wpq0h", FP8, "wpf")
                wp_q0l = load_panel(pwf, wqkl_d, 0, "wpq0l", FP8, "wpf")
        nc.sync.dma_start(out=bqT, in_=bqT_d)
        boB_bcast = bass.AP(
            tensor=boB_d.tensor, offset=boB_d.offset, ap=[[0, 128]] + boB_d.ap
        )
        nc.sync.dma_start(out=boB, in_=boB_bcast)

        # --- VO ones blocks (Pool engine) ---------------------------------
        for kt in range(TT):
            nc.gpsimd.memset(VO[:, kt, :, :, 0:64], 0.03125)

        # --- chain builders ------------------------------------------------
        def kq_chain(wph, wpl, kind, panel, ct, tb):
            """One K/Q projection coltile with W-residual: out [128d, 512t]."""
            gct = panel * 4 + ct
            ps = ppj.tile([128, 512], F32, tag="ps")
            first = True
            for wp in (wph, wpl):
                for eh in range(4):
                    nc.tensor.matmul(
                        ps,
                        lhsT=wp[:, 2 * eh : 2 * eh + 2, ct * 128 : (ct + 1) * 128],
                        rhs=xT8[:, 2 * eh : 2 * eh + 2, tb * 512 : (tb + 1) * 512],
                        start=first,
                        stop=(wp is wpl and eh == 3),
                        perf_mode=PM.DoubleRow,
                    )
                    first = False
            if kind == "k":
                nc.vector.tensor_copy(
                    K8[:, gct, tb * 512 : (tb + 1) * 512], ps
                )
            else:
                nc.vector.tensor_scalar_add(
                    Q8[:, gct, 0, tb * 512 : (tb + 1) * 512],
                    ps,
                    bqT[:, gct : gct + 1],
                )
                nc.vector.scalar_tensor_tensor(
                    out=Q8[:, gct, 1, tb * 512 : (tb + 1) * 512],
                    in0=ps,
                    scalar=bqT[:, gct : gct + 1],
                    in1=Q8[:, gct, 0, tb * 512 : (tb + 1) * 512],
                    op0=mybir.AluOpType.add,
                    op1=mybir.AluOpType.subtract,
                )

        def v_chain(wvp, panel, tt):
            """One V projection token tile (bf16): out [128 tok, 512 vcol],
            split into fp8 hi/lo pairs in the VO windows of hp 4p..4p+3."""
            xtb = pxtb.tile([128, ET, 128], BF16, tag="xtb", name=f"xtb{panel}_{tt}")
            nc.sync.dma_start(
                out=xtb, in_=xTb_r[:, :, tt * 128 : (tt + 1) * 128]
            )
            ps = ppj.tile([128, 512], F32, tag="ps")
            for ee in range(ET):
                nc.tensor.matmul(
                    ps,
                    lhsT=xtb[:, ee, :],
                    rhs=wvp[:, ee, :],
                    start=(ee == 0),
                    stop=(ee == ET - 1),
                )
            ps3 = ps.rearrange("p (hl hd d) -> p hl hd d", hd=2, d=64)
            p0 = panel * 4
            vo_b = VO[:, tt, p0, 0, :]
            hi = bass.AP(
                tensor=vo_b.tensor,
                offset=vo_b.offset + 64,
                ap=[vo_b.ap[0], [512, 4], [256, 2], [1, 64]],
            )
            lo = bass.AP(
                tensor=vo_b.tensor,
                offset=vo_b.offset + 128 + 64,
                ap=[vo_b.ap[0], [512, 4], [256, 2], [1, 64]],
            )
            nc.vector.tensor_copy(hi, ps3)
            nc.vector.tensor_sub(lo, ps3, hi)

        # --- output projection (bf16) --------------------------------------
        wop = [None, None]

        def outproj_chain(tt, half):
            c0 = half * 512
            ps = ppj.tile([128, 512], F32, tag="ps")
            for j in range(4):
                for i in range(2):
                    nc.tensor.matmul(
                        ps,
                        lhsT=AT4[j][:, i, tt * 128 : (tt + 1) * 128],
                        rhs=wop[half][:, 2 * j + i, :],
                        start=(j == 0 and i == 0),
                        stop=(j == 3 and i == 1),
                    )
            osb = p3.tile([128, 512], F32, tag="osb", bufs=2)
            nc.vector.scalar_tensor_tensor(
                out=osb,
                in0=ps,
                scalar=1.0 / 512.0,
                in1=boB[:, c0 : c0 + 512],
                op0=mybir.AluOpType.mult,
                op1=mybir.AluOpType.add,
            )
            nc.sync.dma_start(
                out=out[tt * 128 : (tt + 1) * 128, c0 : c0 + 512], in_=osb
            )

        # --- upfront chains -------------------------------------------------
        for tt in range(6):
            v_chain(wp_v0, 0, tt)
        for tb in range(4):
            kq_chain(wp_k0h, wp_k0l, "k", 0, 0, tb)
        kq_chain(wp_q0h, wp_q0l, "q", 0, 0, 0)

        # --- deferred chain queue (deadline, fn), emitted 1 per kt slot ----
        panels = {}
        deferred = []

        def defer(dl, fn):
            deferred.append((dl, fn))

        for ct in (1, 2, 3):
            for tb in range(4):
                defer(2 * ct, lambda c=ct, t=tb: kq_chain(wp_k0h, wp_k0l, "k", 0, c, t))
        for g in range(1, 4):
            for qb in range(2):
                defer(2 * g + qb, lambda c=g, q=qb: kq_chain(wp_q0h, wp_q0l, "q", 0, c, q))
        defer(1, lambda: kq_chain(wp_q0h, wp_q0l, "q", 0, 0, 1))
        # v1 chains: 6 early (deadlines 6-7), 10 inner in block 8
        for i in range(6):
            defer(6 + i // 3, lambda t=i: v_chain(panels["v1"], 1, t))
        for ct in range(4):
            for tb in range(4):
                defer(8 + 2 * ct, lambda c=ct, t=tb: kq_chain(panels["k1h"], panels["k1l"], "k", 1, c, t))
        for g in range(4):
            for qb in range(2):
                defer(8 + 2 * g + qb, lambda c=g, q=qb: kq_chain(panels["q1h"], panels["q1l"], "q", 1, c, q))
        deferred.sort(key=lambda t: t[0])

        # --- attention ------------------------------------------------------
        att_es = ExitStack()
        pss = att_es.enter_context(tc.tile_pool(name="ps_s", bufs=2, space="PSUM"))
        pav = att_es.enter_context(tc.tile_pool(name="ps_av", bufs=1, space="PSUM"))

        def normalize(hp, qb, av):
            q0 = qb * 512
            rec = p3.tile([64, 1024], F32, tag="rec", bufs=2)
            nc.vector.reciprocal_approx_fast(rec, av[0:64, :])
            j, i = hp // 2, hp % 2
            nc.vector.tensor_mul(
                AT4[j][0:64, i, q0 : q0 + 512], av[64:128, 0:512], rec[:, 0:512]
            )
            nc.vector.tensor_mul(
                AT4[j][64:128, i, q0 : q0 + 512],
                av[64:128, 512:1024],
                rec[:, 512:1024],
            )

        def attention_block(b, hp, qb, inner):
            q0 = qb * 512
            av = pav.tile([128, 1024], F32, tag="av", name=f"av{b}")
            Pt = [None]
            dve_kts = _dve_kts(b)
            pending_av = []

            def emit_av(kt, P):
                for h in range(2):
                    pslot = P[:, h, kt % 2, :]
                    rhs = bass.AP(
                        tensor=pslot.tensor,
                        offset=pslot.offset,
                        ap=[pslot.ap[0], [0, 2], pslot.ap[-1]],
                    )
                    nc.tensor.matmul(
                        av[:, h * 512 : (h + 1) * 512],
                        lhsT=VO[:, kt, hp, 2 * h : 2 * h + 2, :],
                        rhs=rhs,
                        start=(kt == 0),
                        stop=(kt == TT - 1),
                        perf_mode=PM.DoubleRow,
                    )

            for kt in range(TT):
                for fn in inner.get(kt, ()):
                    fn()
                ktm = kt % 2
                if ktm == 0:
                    Pt[0] = pP.tile(
                        [128, 2, 2, 512], FP8, tag="P", name=f"P{b}_{kt}"
                    )
                P = Pt[0]
                ps = pss.tile([128, 1024], F32, tag="ps_s")
                for h in range(2):
                    r0 = 64 * h
                    kbase = K8[r0 : r0 + 64, hp, kt * 128 : (kt + 1) * 128]
                    lhsT = bass.AP(
                        tensor=kbase.tensor,
                        offset=kbase.offset,
                        ap=[kbase.ap[0], [0, 2], kbase.ap[-1]],
                    )
                    nc.tensor.matmul(
                        ps[:, h * 512 : (h + 1) * 512],
                        lhsT=lhsT,
                        rhs=Q8[r0 : r0 + 64, hp, :, q0 : q0 + 512],
                        start=True,
                        stop=True,
                        perf_mode=PM.DoubleRow,
                    )
                if kt in dve_kts:
                    nc.vector.tensor_scalar(
                        out=P[:, :, ktm, :].bitcast(I8),
                        in0=ps,
                        scalar1=C1,
                        scalar2=C2,
                        op0=mybir.AluOpType.mult,
                        op1=mybir.AluOpType.add,
                    )
                else:
                    nc.scalar.activation(P[:, :, ktm, :], ps, AF.Exp, scale=SC)
                pending_av.append((kt, P))
                if len(pending_av) > 1:
                    emit_av(*pending_av.pop(0))
            for item in pending_av:
                emit_av(*item)
            return av

        blocks = [(hp, qb) for hp in range(HP) for qb in range(QB)]
        av_prev = None
        di = 0
        for b, (hp, qb) in enumerate(blocks):
            if av_prev is not None:
                normalize(*av_prev)
                av_prev = None
            if b == 3:
                panels["v1"] = load_panel(pwb, wvb_d, 512, "wpv1", BF16, "wpb")
            if b == 5:
                panels["k1h"] = load_panel(pwf, wqkh_d, E + 512, "wpk1h", FP8, "wpf")
                panels["k1l"] = load_panel(pwf, wqkl_d, E + 512, "wpk1l", FP8, "wpf")
            if b == 6:
                panels["q1h"] = load_panel(pwf, wqkh_d, 512, "wpq1h", FP8, "wpf")
                panels["q1l"] = load_panel(pwf, wqkl_d, 512, "wpq1l", FP8, "wpf")
            if b == 13:
                # reuse the bf16 panel pool (rotation deps make this safe)
                wop[0] = load_panel(pwb, wob_d, 0, "wpo0", BF16, "wpb")
                wop[1] = load_panel(pwb, wob_d, 512, "wpo1", BF16, "wpb")

            # schedule this block's inner work: 1 item per kt slot
            inner = {}
            slot_fns = []
            if b == 0:
                slot_fns += [
                    (lambda t=tt: v_chain(wp_v0, 0, t)) for tt in range(6, TT)
                ]
            if b == 8:
                slot_fns += [
                    (lambda t=tt: v_chain(panels["v1"], 1, t))
                    for tt in range(6, TT)
                ]
            while di < len(deferred) and deferred[di][0] <= b + 1:
                slot_fns.append(deferred[di][1])
                di += 1
            if b == 15:
                for tt in range(4):
                    for half in range(2):
                        slot_fns.append(lambda t=tt, hf=half: outproj_chain(t, hf))
            start = 2 if b > 0 else 0
            for i, fn in enumerate(slot_fns):
                inner.setdefault(min(start + i, TT - 1), []).append(fn)

            av = attention_block(b, hp, qb, inner)
            av_prev = (hp, qb, av)
            if b >= 14:
                # normalize immediately: block 15's inner outproj chains (qb0)
                # and the tail (qb1) read these AT4 columns
                normalize(*av_prev)
                av_prev = None
        assert di == len(deferred), (di, len(deferred))
        for tt in range(4, 8):
            for half in range(2):
                outproj_chain(tt, half)
        att_es.close()


_NC = None


def _get_program():
    global _NC
    if _NC is None:
        _NC = build_program()
    return _NC


def make_in_maps(x, Wqkv, bqkv, Wo, bo):
    np8 = mybir.dt.np(FP8)
    npb = mybir.dt.np(BF16)
    Wqkv = np.asarray(Wqkv, np.float32)
    Wo = np.asarray(Wo, np.float32)
    bqkv = np.asarray(bqkv, np.float32)
    bo = np.asarray(bo, np.float32)
    wqk32 = 32.0 * Wqkv[:, 0 : 2 * E]
    wqk_hi = wqk32.astype(np8)
    wqk_lo = (wqk32 - wqk_hi.astype(np.float32)).astype(np8)
    w = {
        "wqk_hi": np.ascontiguousarray(wqk_hi),
        "wqk_lo": np.ascontiguousarray(wqk_lo),
        "wv_b": np.ascontiguousarray((32.0 * Wqkv[:, 2 * E :]).astype(npb)),
        "wo_b": np.ascontiguousarray(Wo.astype(npb)),
        "bqT": np.ascontiguousarray(
            (32.0 * bqkv[0:E]).reshape(ET, 128).T.astype(np.float32)
        ),
        "boB": np.ascontiguousarray((bqkv[2 * E :] @ Wo + bo).astype(np.float32)),
    }
    x = np.asarray(x, np.float32)
    in_maps = []
    for c in range(N_CORES):
        b, s = divmod(c, 2)
        xb = x[b]
        if s == 1:
            xb = np.roll(xb, -NQ, axis=0)
        xt = np.ascontiguousarray(xb.T)
        in_maps.append(
            {
                "xT8": np.ascontiguousarray(xt.astype(np8)),
                "xTb": np.ascontiguousarray(xt.astype(npb)),
                **w,
            }
        )
    return in_maps


def gather_out(results):
    out = np.empty((4, SEQ, E), np.float32)
    for c in range(N_CORES):
        b, s = divmod(c, 2)
        out[b, s * NQ : (s + 1) * NQ] = results[c]["out"]
    return out


def kernel(x, Wqkv, bqkv, Wo, bo):
    from concourse.bass_utils import run_bass_kernel_spmd

    nc = _get_program()
    in_maps = make_in_maps(x, Wqkv, bqkv, Wo, bo)
    res = run_bass_kernel_spmd(nc, in_maps, core_ids=list(range(N_CORES)))
    return gather_out(res.results)
